# revision 14
# baseline (speedup 1.0000x reference)
"""CAB multi-head attention on 8 Trainium2 NeuronCores.

Sharding: fully data-parallel, core c -> (batch b = c//2, query-half = c%2).
Each core computes 256 query rows against all 512 keys of its batch.
No collectives. Host does transposes/packing; device does all FLOPs.

Algebraic reductions exploited (all exact w.r.t. softmax):
  - bk drops out (adds a per-query constant to every logit row),
  - b3/head_temps offset drops out (per-head constant), temps fold into W3,
  - bv contributes bv @ Wo to every output row (softmax rows sum to 1),
    added host-side together with bo.

Per-core layout (features on partitions, tokens on free):
  QT/KT [E, t] bf16; V65 [j, 8*(64+1)] bf16 with a ones column per head
  slice so the AV matmul also produces the softmax denominator;
  scoresT [j, i] per (head, jc-pair) with CAB bias preloaded into PSUM
  via an identity matmul; CAB pair MLP packs two i per 128 partitions
  (block-diagonal W2/W3).

Scheduling: W1 runs first; the Q/K/V projection matmuls are interleaved
into the 32-iteration CAB loop (one unit per iteration) so the PE never
waits on the vector/scalar engines producing h/h2; W3+transpose trail
one iteration behind W2 (software pipeline). All elementwise work runs
on DVE + ACT (GPSIMD is ~20x slower per op and is not used).
"""
import sys

sys.path.insert(0, "/opt/trn_rl_repo")

import numpy as np
import ml_dtypes
from contextlib import ExitStack

import concourse.bacc as bacc
import concourse.tile as tile
from concourse import mybir
from concourse.bass_utils import run_bass_kernel_spmd

F32 = mybir.dt.float32
F32R = mybir.dt.float32r
BF16 = mybir.dt.bfloat16
AF = mybir.ActivationFunctionType
ALU = mybir.AluOpType

B, N, E, H, SD, HID = 4, 512, 1024, 16, 64, 64
D = E // H
NQ = 256            # query rows per core
NCORES = 8
NTT = NQ // 8       # 32 tt groups (4 i-pairs each) in the CAB stage

_BF = ml_dtypes.bfloat16


def _build_program(debug=False):
    nc = bacc.Bacc("TRN2", target_bir_lowering=False, debug=False,
                   num_devices=NCORES)

    def din(name, shape, dt):
        return nc.dram_tensor(name, list(shape), dt, kind="ExternalInput").ap()

    d = {}
    d["qT"] = din("qT", (E, NQ), BF16)
    d["kT"] = din("kT", (E, N), BF16)
    d["seT"] = din("seT", (SD, N), F32R)
    d["seQ"] = din("seQ", (SD, NQ), F32R)
    d["wqp"] = din("wqp", (8, 128, 1024), BF16)   # [ec][p][(k c)]
    d["wkp"] = din("wkp", (8, 128, 1024), BF16)
    d["vtp"] = din("vtp", (4, 128, 1024), BF16)   # [st][p][(k c)]
    d["wv"] = din("wv", (E, E), BF16)
    d["wo"] = din("wo", (E, E), BF16)
    d["w1a"] = din("w1a", (SD, 128), F32R)
    d["w1b"] = din("w1b", (SD, 128), F32R)
    d["w2bd"] = din("w2bd", (128, 128), BF16)
    d["w3bd"] = din("w3bd", (128, 32), BF16)
    d["id128"] = din("id128", (128, 128), BF16)
    d["bq128"] = din("bq128", (128, 8), F32)
    d["b1d"] = din("b1d", (128, 1), F32)
    d["b2d"] = din("b2d", (128, 1), F32)
    d["ones8"] = din("ones8", (128, 8), BF16)
    out_d = nc.dram_tensor("out", [NQ, E], F32, kind="ExternalOutput").ap()
    sscratch = nc.dram_tensor("sscratch", [16, NQ], F32).ap()
    rscratch = nc.dram_tensor("rscratch", [16, NQ], F32).ap()
    dbg = {}
    if debug:
        for nm, shape, dt in [
                ("dQT", [128, NQ], BF16), ("dKT", [128, N], BF16),
                ("dV65", [128, 520], BF16), ("dbias", [128, NTT * 512], BF16),
                ("dsums", [16, NQ], F32), ("drecip", [16, NQ], F32),
                ("davU", [128, NQ], F32), ("dhjT", [128, N], BF16),
                ("dhiT", [128, 128], F32)]:
            dbg[nm] = nc.dram_tensor(nm, shape, dt,
                                     kind="ExternalOutput").ap()

    with tile.TileContext(nc) as tc, ExitStack() as ctx:
        # ---------------- persistent SBUF pools ----------------
        cst = ctx.enter_context(tc.tile_pool(name="cst", bufs=1))
        big = ctx.enter_context(tc.tile_pool(name="big", bufs=1))

        def cload(name, shape, dt):
            t = cst.tile(list(shape), dt, tag=name, name=name)
            nc.sync.dma_start(t[:], d[name][:])
            return t

        # small/critical constants first: W1 path gates everything
        w1a = cload("w1a", (SD, 128), F32R)
        w1b = cload("w1b", (SD, 128), F32R)
        seT = cload("seT", (SD, N), F32R)
        seQ = cload("seQ", (SD, NQ), F32R)
        b1d = cload("b1d", (128, 1), F32)
        b2d = cload("b2d", (128, 1), F32)
        id128 = cload("id128", (128, 128), BF16)
        w2bd = cload("w2bd", (128, 128), BF16)
        w3bd = cload("w3bd", (128, 32), BF16)
        bq128 = cload("bq128", (128, 8), F32)
        ones8 = cload("ones8", (128, 8), BF16)

        # resident per-core inputs, chunked on k
        def kchunks(name, t, dt, ntile=8):
            ts = []
            for k in range(ntile):
                tt = big.tile([128, t], dt, tag=f"{name}{k}", name=f"{name}{k}")
                nc.sync.dma_start(tt[:], d[name][k * 128:(k + 1) * 128, :])
                ts.append(tt)
            return ts

        qTt = kchunks("qT", NQ, BF16)
        kTt = kchunks("kT", N, BF16)
        wv_r = kchunks("wv", E, BF16)
        wo_r = kchunks("wo", E, BF16)

        # persistent intermediates
        QT = [big.tile([128, NQ], BF16, tag=f"QT{k}", name=f"QT{k}") for k in range(8)]
        KT = [big.tile([128, N], BF16, tag=f"KT{k}", name=f"KT{k}") for k in range(8)]
        # V65: per (st, et): [j 128, 8 head-slices x (64 d + ones)]
        V65 = [[big.tile([128, 8 * 65], BF16, tag=f"V{st}_{et}", name=f"V{st}_{et}")
                for et in range(2)] for st in range(4)]
        hjT = big.tile([128, N], BF16, tag="hjT")
        hiT = big.tile([128, 128], F32, tag="hiT")
        biasT = big.tile([128, NTT * 512], BF16, tag="biasT")
        avU = [big.tile([128, NQ], F32, tag=f"avU{hp}", name=f"avU{hp}") for hp in range(8)]
        avN = [big.tile([128, NQ], BF16, tag=f"avN{hp}", name=f"avN{hp}") for hp in range(8)]
        sums_flat = big.tile([1, 16 * NQ], F32, tag="sums_flat")
        sums_sb = big.tile([16, NQ], F32, tag="sums_sb")
        recip_sb = big.tile([16, NQ], F32, tag="recip_sb")

        # ones columns of V65 (disjoint from the projection copy below)
        for st in range(4):
            for et in range(2):
                v65v = V65[st][et][:].rearrange("p (h c) -> p h c", c=65)
                nc.sync.dma_start(v65v[:, :, 64:65],
                                  d["ones8"][:].rearrange("p (h o) -> p h o",
                                                          o=1))

        # ---------------- fused phase 1+2 ----------------
        with tc.tile_pool(name="wcol", bufs=3) as wcol, \
             tc.tile_pool(name="hpool", bufs=8) as hpool, \
             tc.tile_pool(name="h2sb", bufs=8) as h2sbp, \
             tc.tile_pool(name="csb", bufs=2) as csbp, \
             tc.tile_pool(name="p1ps", bufs=2, space="PSUM") as p1ps, \
             tc.tile_pool(name="h2ps", bufs=2, space="PSUM") as h2ps, \
             tc.tile_pool(name="cps", bufs=2, space="PSUM") as cps, \
             tc.tile_pool(name="trps", bufs=2, space="PSUM") as trps:

            # W1 first: everything in the CAB loop hangs off hjT/hiT.
            hj_ps = p1ps.tile([128, N], F32, tag="p1", name="hjps")
            nc.tensor.matmul(hj_ps[:], w1b[:], seT[:], start=True, stop=True)
            nc.vector.tensor_scalar(hjT[:], hj_ps[:], b1d[:, 0:1], None,
                                    ALU.add)
            hi_ps = p1ps.tile([128, NQ], F32, tag="p1", name="hips")
            nc.tensor.matmul(hi_ps[:], w1a[:], seQ[:], start=True, stop=True)
            hi_v = hi_ps[:].rearrange("p (i two) -> p i two", two=2)
            nc.vector.tensor_copy(hiT[0:64, :], hi_v[0:64, :, 0])
            nc.vector.tensor_copy(hiT[64:128, :], hi_v[64:128, :, 1])

            # projection units, one interleaved per CAB iteration
            def q_unit(ec):
                wq_c = wcol.tile([128, 1024], BF16, tag="wcol")
                nc.sync.dma_start(wq_c[:], d["wqp"][ec])
                ps = p1ps.tile([128, 512], F32, tag="p1", name="qps")[:, 0:NQ]
                for kc in range(8):
                    nc.tensor.matmul(ps[:], wq_c[:, kc * 128:(kc + 1) * 128],
                                     qTt[kc][:], start=(kc == 0),
                                     stop=(kc == 7))
                nc.vector.tensor_scalar(QT[ec][:], ps[:],
                                        bq128[:, ec:ec + 1], None, ALU.add)

            def k_unit(ec):
                wk_c = wcol.tile([128, 1024], BF16, tag="wcol")
                nc.sync.dma_start(wk_c[:], d["wkp"][ec])
                ps = p1ps.tile([128, 512], F32, tag="p1", name="kps")
                for kc in range(8):
                    nc.tensor.matmul(ps[:], wk_c[:, kc * 128:(kc + 1) * 128],
                                     kTt[kc][:], start=(kc == 0),
                                     stop=(kc == 7))
                nc.vector.tensor_copy(KT[ec][:], ps[:])

            def v_unit(st, et):
                if et == 0:
                    vt_c = wcol.tile([128, 1024], BF16, tag=f"vcol{st}")
                    nc.sync.dma_start(vt_c[:], d["vtp"][st])
                    v_unit.cache[st] = vt_c
                vt_c = v_unit.cache[st]
                ps = p1ps.tile([128, 512], F32, tag="p1", name="vps")
                for kc in range(8):
                    nc.tensor.matmul(
                        ps[:], vt_c[:, kc * 128:(kc + 1) * 128],
                        wv_r[kc][:, et * 512:(et + 1) * 512],
                        start=(kc == 0), stop=(kc == 7))
                dstv = V65[st][et][:].rearrange("p (h c) -> p h c", c=65)
                nc.vector.tensor_copy(
                    dstv[:, :, 0:64],
                    ps[:].rearrange("p (h c) -> p h c", c=64))
            v_unit.cache = {}

            units = ([lambda ec=ec: k_unit(ec) for ec in range(8)]
                     + [lambda st=st, et=et: v_unit(st, et)
                        for st in range(4) for et in range(2)]
                     + [lambda ec=ec: q_unit(ec) for ec in range(8)])

            # CAB software pipeline: stage A (h_t -> W2 -> h2) at tt,
            # stage B (W3 -> transpose -> biasT) at tt-1.
            h2_hist = {}

            def stage_a(tt):
                h2_tiles = []
                for iic in range(4):
                    ii = tt * 4 + iic
                    h_t = hpool.tile([128, N], BF16, tag="h")
                    nc.vector.tensor_scalar(h_t[:], hjT[:], hiT[:, ii:ii + 1],
                                            0.0, ALU.add, ALU.max)
                    ps = h2ps.tile([128, N], F32, tag="h2")
                    nc.tensor.matmul(ps[:], w2bd[:], h_t[:], start=True,
                                     stop=True)
                    h2_t = h2sbp.tile([128, N], BF16, tag="h2sb")
                    nc.scalar.activation(h2_t[:], ps[:], AF.Relu,
                                         bias=b2d[:, 0:1])
                    h2_tiles.append(h2_t)
                h2_hist[tt] = h2_tiles

            def stage_b(tt):
                h2_tiles = h2_hist.pop(tt)
                c_ps = cps.tile([128, N], F32, tag="comp")
                for iic in range(4):
                    nc.tensor.matmul(c_ps[32 * iic:32 * iic + 32, :],
                                     w3bd[:], h2_tiles[iic][:],
                                     start=True, stop=True,
                                     tile_position=(0, 32 * iic))
                c_sb = csbp.tile([128, N], BF16, tag="csb")
                nc.vector.tensor_copy(c_sb[:], c_ps[:])
                tr_ps = trps.tile([128, 512], BF16, tag="tr")
                for jc in range(4):
                    nc.tensor.transpose(tr_ps[:, jc * 128:(jc + 1) * 128],
                                        c_sb[:, jc * 128:(jc + 1) * 128],
                                        id128[:])
                nc.vector.tensor_copy(biasT[:, tt * 512:(tt + 1) * 512],
                                      tr_ps[:])

            for tt in range(NTT):
                if tt < len(units):
                    units[tt]()
                stage_a(tt)
                if tt >= 1:
                    stage_b(tt - 1)
            for u in range(NTT, len(units)):
                units[u]()
            stage_b(NTT - 1)

        # ---------------- phase 3: scores + softmax + AV ----------------
        with tc.tile_pool(name="attnT", bufs=4) as attp, \
             tc.tile_pool(name="scps", bufs=3, space="PSUM") as scps, \
             tc.tile_pool(name="avps", bufs=2, space="PSUM") as avps, \
             tc.tile_pool(name="r2sb", bufs=2) as r2sb:

            bview5 = biasT[:].rearrange("p (t j i m x) -> p t j i m x",
                                        t=NTT, j=4, i=4, m=2, x=16)
            for h in range(16):
                hp, hw = h // 2, (h % 2) * 64
                av_ps = avps.tile([65, NQ], F32, tag="av")
                for jcp in range(2):
                    sc_ps = scps.tile([128, 512], F32, tag="sc")
                    for j2 in range(2):
                        jc = 2 * jcp + j2
                        sl = sc_ps[:, j2 * NQ:(j2 + 1) * NQ]
                        nc.tensor.matmul(sl, id128[:],
                                         bview5[:, :, jc, :, :, h],
                                         start=True, stop=False,
                                         skip_group_check=True)
                        nc.tensor.matmul(
                            sl,
                            KT[hp][hw:hw + 64, jc * 128:(jc + 1) * 128],
                            QT[hp][hw:hw + 64, :],
                            start=False, stop=True, skip_group_check=True)
                    at2 = attp.tile([128, 512], BF16, tag="at")
                    nc.scalar.activation(at2[:], sc_ps[:], AF.Exp)
                    for j2 in range(2):
                        jc = 2 * jcp + j2
                        nc.tensor.matmul(
                            av_ps[:],
                            V65[jc][h // 8][:, (h % 8) * 65:(h % 8) * 65 + 65],
                            at2[:, j2 * NQ:(j2 + 1) * NQ],
                            start=(jc == 0), stop=(jc == 3),
                            skip_group_check=True)
                nc.vector.tensor_copy(avU[hp][hw:hw + 64, :], av_ps[0:64, :])
                # engines can't write at partition offset h (32-aligned only):
                # stage the denominator rows side by side on partition 0,
                # then reshape to [16, NQ] via a DRAM bounce
                nc.vector.tensor_copy(sums_flat[0:1, h * NQ:(h + 1) * NQ],
                                      av_ps[64:65, :])

            # reshape happens on the DRAM side (linear); an SBUF AP cannot
            # fold free elements into the physical partition axis
            nc.sync.dma_start(
                sscratch[:].rearrange("(o h) t -> o (h t)", o=1),
                sums_flat[0:1, :])
            nc.sync.dma_start(sums_sb[:], sscratch[:])
            nc.vector.reciprocal(recip_sb[:], sums_sb[:])
            if dbg:
                nc.sync.dma_start(dbg["dQT"][:], QT[0][:])
                nc.sync.dma_start(dbg["dKT"][:], KT[0][:])
                nc.sync.dma_start(dbg["dV65"][:], V65[0][0][:])
                nc.sync.dma_start(dbg["dbias"][:], biasT[:])
                nc.sync.dma_start(dbg["dsums"][:], sums_sb[:])
                nc.sync.dma_start(dbg["drecip"][:], recip_sb[:])
                nc.sync.dma_start(dbg["davU"][:], avU[0][:])
                nc.sync.dma_start(dbg["dhjT"][:], hjT[:])
                nc.sync.dma_start(dbg["dhiT"][:], hiT[:])
            nc.sync.dma_start(rscratch[:], recip_sb[:])
            for hp in range(8):
                r2 = r2sb.tile([128, NQ], F32, tag="r2")
                rsrc = rscratch[2 * hp:2 * hp + 2, :].rearrange(
                    "h (o t) -> h o t", o=1)
                nc.sync.dma_start(r2[:], rsrc.broadcast_to([2, 64, NQ]))
                nc.vector.tensor_tensor(avN[hp][:], avU[hp][:], r2[:],
                                        ALU.mult)

        # ---------------- phase 4: output projection ----------------
        with tc.tile_pool(name="osb", bufs=2) as osb, \
             tc.tile_pool(name="ops", bufs=2, space="PSUM") as ops:
            for ttile in range(2):
                for et in range(2):
                    ps = ops.tile([128, 512], F32, tag="ops")
                    for hp in range(8):
                        nc.tensor.matmul(
                            ps[:], avN[hp][:, ttile * 128:(ttile + 1) * 128],
                            wo_r[hp][:, et * 512:(et + 1) * 512],
                            start=(hp == 0), stop=(hp == 7))
                    o_sb = osb.tile([128, 512], F32, tag="osb")
                    nc.scalar.copy(o_sb[:], ps[:])
                    nc.sync.dma_start(
                        out_d[ttile * 128:(ttile + 1) * 128,
                              et * 512:(et + 1) * 512], o_sb[:])

    nc.compile()
    return nc


def _host_prep(inputs):
    """Build the 8 per-core input maps from the full inputs."""
    f32 = np.float32
    q = np.ascontiguousarray(inputs["query"], f32)
    k = np.ascontiguousarray(inputs["key"], f32)
    v = np.ascontiguousarray(inputs["value"], f32)
    se = np.ascontiguousarray(inputs["state_embeddings"], f32)
    scale = f32(D) ** f32(-0.5)
    wq = np.ascontiguousarray(inputs["Wq"] * scale, f32)
    wk = np.ascontiguousarray(inputs["Wk"], f32)
    wv = np.ascontiguousarray(inputs["Wv"], f32)
    wo = np.ascontiguousarray(inputs["Wo"], f32)
    bq = np.asarray(inputs["bq"], f32) * scale
    w1 = np.asarray(inputs["W1"], f32)
    b1 = np.asarray(inputs["b1"], f32)
    w2 = np.asarray(inputs["W2"], f32)
    b2 = np.asarray(inputs["b2"], f32)
    w3 = np.asarray(inputs["W3"], f32)
    temps = np.asarray(inputs["head_temps"], f32)

    # [ec][p][(k c)] packs: element [ec, p, k*128+c] = w[k*128+p, ec*128+c]
    wqp = wq.reshape(8, 128, 8, 128).transpose(2, 1, 0, 3).reshape(8, 128, 1024)
    wkp = wk.reshape(8, 128, 8, 128).transpose(2, 1, 0, 3).reshape(8, 128, 1024)

    w1a_dup = np.concatenate([w1[:SD], w1[:SD]], axis=1)          # [64,128]
    w1b_dup = np.concatenate([w1[SD:], w1[SD:]], axis=1)          # [64,128]
    w2bd = np.zeros((128, 128), f32)
    w2bd[:64, :64] = w2
    w2bd[64:, 64:] = w2
    w3t = w3 * temps[None, :]                                     # [64,16]
    w3bd = np.zeros((128, 32), f32)
    w3bd[:64, :16] = w3t
    w3bd[64:, 16:] = w3t
    b1d = np.tile(b1, 2).reshape(128, 1)
    b2d = np.tile(b2, 2).reshape(128, 1)
    bq128 = bq.reshape(8, 128).T.copy()
    id128 = np.eye(128, dtype=f32).astype(_BF)
    ones8 = np.ones((128, 8), f32).astype(_BF)

    shared = dict(wqp=wqp.astype(_BF), wkp=wkp.astype(_BF),
                  wv=wv.astype(_BF), wo=wo.astype(_BF),
                  w1a=w1a_dup, w1b=w1b_dup,
                  w2bd=w2bd.astype(_BF), w3bd=w3bd.astype(_BF),
                  id128=id128, ones8=ones8, bq128=bq128,
                  b1d=b1d, b2d=b2d)
    maps = []
    for c in range(NCORES):
        b, half = c // 2, c % 2
        rows = slice(half * NQ, (half + 1) * NQ)
        m = dict(shared)
        m["qT"] = np.ascontiguousarray(q[b, rows].T).astype(_BF)
        m["kT"] = np.ascontiguousarray(k[b].T).astype(_BF)
        # [st][p][(k c)]: element [st, p, k*128+c] = v[b, st*128+c, k*128+p]
        m["vtp"] = np.ascontiguousarray(
            v[b].reshape(4, 128, 8, 128).transpose(0, 3, 2, 1)
            .reshape(4, 128, 1024)).astype(_BF)
        m["seT"] = np.ascontiguousarray(se[b].T)
        m["seQ"] = np.ascontiguousarray(se[b, rows].T)
        maps.append(m)
    return maps


_cache = {}


def _get_program():
    if "nc" not in _cache:
        _cache["nc"] = _build_program()
    return _cache["nc"]


def kernel(**inputs):
    nc = _get_program()
    maps = _host_prep(inputs)
    res = run_bass_kernel_spmd(nc, maps, list(range(NCORES)))
    # softmax rows sum to 1, so bv contributes exactly bv @ Wo to every row
    f32 = np.float32
    off = (np.asarray(inputs["bv"], f32) @ np.asarray(inputs["Wo"], f32)
           + np.asarray(inputs["bo"], f32))
    out = np.empty((B, N, E), np.float32)
    for c in range(NCORES):
        b, half = c // 2, c % 2
        out[b, half * NQ:(half + 1) * NQ] = res.results[c]["out"]
    return out + off
